# revision 27
# baseline (speedup 1.0000x reference)
"""Trainium2 Bass kernel for nn_MiniAttn (B=2, N=8192, D=768 attention w/ S2 rope).

Sharding: 8 cores = 2 batches x 4 q-chunks of 2048 rows. Each core gets its
batch's tokens *rolled* so its q-chunk is always rows 0:2048 (softmax over
keys is permutation invariant), letting one SPMD program serve all cores.

Device algorithm (per core):
  phase A: cast x/W to fp16 in DRAM, DMA-transpose weights to SBUF,
           compute psi=arccos(vdn.refn) on-chip, build theta/phi/psi rows.
  phase B: per 512-token block: DMA-transpose x^T slice, project K^T/Q^T/V
           (fp16 matmuls), apply rope via sin/cos built from PE outer
           products + ACT Sin; K_rot^T and Q_rot^T stay resident in SBUF,
           V streams to DRAM.
  phase C: flash-attention without max-subtraction (scores ~N(0,1)):
           S^T tile = K_rot^T.T @ Q_rot^T, P=exp(S/sqrt(D)) (ACT, fp16),
           U^T += V.T @ P per k-tile (PSUM accum), l += P.T @ ones,
           then out = (U^T.T @ Wo^T) * (1/l) + bo.
"""

import math
import sys

import numpy as np

sys.path.insert(0, "/opt/trn_rl_repo")

import concourse.bass as bass  # noqa: E402
import concourse.mybir as mybir  # noqa: E402
import concourse.tile as tile  # noqa: E402
from concourse import bacc  # noqa: E402
from concourse.bass_utils import run_bass_kernel_spmd  # noqa: E402

F32 = mybir.dt.float32
F16 = mybir.dt.float16
F32R = mybir.dt.float32r

B, N, D = 2, 8192, 768
NCORES = 8
SPLIT = NCORES // B          # q-chunks per batch
CHUNK = N // SPLIT           # q rows per core
ROPE_BASE = 10000.0
NPAIR = D // 2               # 384 rotary pairs
PER = NPAIR // 3             # 128 pairs per angle channel
OC = D // 128                # 6 output 128-chunks
TB = 512                     # token block in phase B
SCALE = 1.0 / math.sqrt(D)

# arccos(x) ~= sqrt(1-x) * poly(x) on [0,1]  (Abramowitz & Stegun 4.4.46,
# |err| <= 2e-8); arccos(-x) = pi - arccos(x)
ACOS_C = [1.5707963050, -0.2145988016, 0.0889789874, -0.0501743046,
          0.0308918810, -0.0170881256, 0.0066700901, -0.0012624911]


def build_nc(n=N, chunk=CHUNK):
    assert n % TB == 0 and chunk % 128 == 0
    qb_size = min(512, chunk)
    n_qb = chunk // qb_size
    n_tb = n // TB
    n_kt = n // 128
    nrows = n // TB          # partitions in the [nrows, TB] angle-row tiles

    nc = bacc.Bacc(None)

    x = nc.declare_dram_parameter("x", [n, D], F32, isOutput=False)
    vdp = nc.declare_dram_parameter("vdp", [128, (n // 128) * 3], F32, isOutput=False)
    posp = nc.declare_dram_parameter("posp", [2, nrows, TB], F32, isOutput=False)
    ref = nc.declare_dram_parameter("ref", [1, 3], F32, isOutput=False)
    wq = nc.declare_dram_parameter("Wq", [D, D], F32, isOutput=False)
    wk = nc.declare_dram_parameter("Wk", [D, D], F32, isOutput=False)
    wv = nc.declare_dram_parameter("Wv", [D, D], F32, isOutput=False)
    wo = nc.declare_dram_parameter("Wo", [D, D], F32, isOutput=False)
    bqc = nc.declare_dram_parameter("bqc", [128, OC], F32, isOutput=False)
    bkc = nc.declare_dram_parameter("bkc", [128, OC], F32, isOutput=False)
    bvr = nc.declare_dram_parameter("bvr", [1, D], F32, isOutput=False)
    bor = nc.declare_dram_parameter("bor", [1, D], F32, isOutput=False)
    invf = nc.declare_dram_parameter("invf", [1, PER], F32, isOutput=False)
    out = nc.declare_dram_parameter("out", [chunk, D], F32, isOutput=True)

    ws = [wq, wk, wv, wo]

    with tile.TileContext(nc) as tc:
        with (
            tc.tile_pool(name="dram", bufs=1, space="DRAM") as dram,
            tc.tile_pool(name="consts", bufs=1) as consts,
            tc.tile_pool(name="resident", bufs=1) as res,
        ):
            # ---- DRAM scratch ----
            x16d = dram.tile([n, D], F16)
            w16d = dram.tile([4, D, D], F16)
            v16d = dram.tile([n, D], F16)
            rowsd = dram.tile([3, n], F32)

            # fp32 -> fp16 casts (SWDGE DMA casts in flight)
            nc.gpsimd.dma_start(out=x16d, in_=x[:])
            for i in range(4):
                nc.gpsimd.dma_start(out=w16d[i], in_=ws[i][:])

            # ---- weights transposed into SBUF: wt[w][:, ic, :] = W^T[in-chunk ic]
            wt = []
            for wi in range(4):
                wtile = res.tile([128, OC, D], F16, name=f"wt{wi}")
                for ic in range(OC):
                    nc.sync.dma_start(
                        out=wtile[:, ic, :],
                        in_=w16d[wi, :, ic * 128:(ic + 1) * 128],
                        transpose=True,
                    )
                wt.append(wtile)

            # ---- small consts ----
            bq_s = consts.tile([128, OC], F32)
            nc.sync.dma_start(out=bq_s, in_=bqc[:])
            bk_s = consts.tile([128, OC], F32)
            nc.sync.dma_start(out=bk_s, in_=bkc[:])
            bv_b = consts.tile([128, D], F32)
            nc.gpsimd.dma_start(out=bv_b, in_=bvr[:].broadcast_to([128, D]))
            invf_s = consts.tile([1, PER], F32)
            nc.sync.dma_start(out=invf_s, in_=invf[:])
            ones16 = consts.tile([128, 1], F16)
            nc.vector.memset(ones16, 1.0)
            halfpi = consts.tile([128, 1], F32)
            nc.vector.memset(halfpi, math.pi / 2)

            # ---- psi = arccos(clip(vdn . refn)) in [128, n/128] p-major layout
            FF = n // 128
            with tc.tile_pool(name="psip", bufs=1) as pp:
                r13 = pp.tile([1, 3], F32)
                nc.sync.dma_start(out=r13, in_=ref[:])
                rsq = pp.tile([1, 3], F32)
                nc.vector.tensor_mul(rsq, r13, r13)
                rs1 = pp.tile([1, 1], F32)
                nc.vector.reduce_sum(rs1, rsq, axis=mybir.AxisListType.X)
                rsr = pp.tile([1, 1], F32)
                nc.scalar.sqrt(rsr, rs1)
                rrec = pp.tile([1, 1], F32)
                nc.vector.reciprocal(rrec, rsr)
                r13n = pp.tile([1, 3], F32)
                nc.vector.tensor_scalar(r13n, r13, rrec, None, mybir.AluOpType.mult)
                refd = dram.tile([1, 3], F32)
                nc.sync.dma_start(out=refd, in_=r13n)
                refb = pp.tile([128, 3], F32)
                nc.gpsimd.dma_start(out=refb, in_=refd[:].broadcast_to([128, 3]))

                vdt = pp.tile([128, FF, 3], F32)
                nc.sync.dma_start(out=vdt, in_=vdp[:].rearrange("p (f c) -> p f c", c=3))
                sq3 = pp.tile([128, FF, 3], F32)
                nc.vector.tensor_mul(sq3, vdt, vdt)
                nrm = pp.tile([128, FF], F32)
                nc.vector.reduce_sum(nrm, sq3, axis=mybir.AxisListType.X)
                srt = pp.tile([128, FF], F32)
                nc.scalar.sqrt(srt, nrm)
                rsn = pp.tile([128, FF], F32)
                nc.vector.reciprocal(rsn, srt)
                prod = pp.tile([128, FF, 3], F32)
                nc.vector.tensor_mul(
                    prod, vdt, refb[:, None, :].broadcast_to([128, FF, 3])
                )
                dotv = pp.tile([128, FF], F32)
                nc.vector.reduce_sum(dotv, prod, axis=mybir.AxisListType.X)
                cc = pp.tile([128, FF], F32)
                nc.vector.tensor_mul(cc, dotv, rsn)
                nc.vector.tensor_scalar_min(cc, cc, 1.0)
                nc.vector.tensor_scalar_max(cc, cc, -1.0)
                av = pp.tile([128, FF], F32)
                nc.scalar.activation(av, cc, mybir.ActivationFunctionType.Abs)
                onema = pp.tile([128, FF], F32)
                nc.vector.tensor_scalar(
                    onema, av, -1.0, 1.0, mybir.AluOpType.mult, mybir.AluOpType.add
                )
                sr = pp.tile([128, FF], F32)
                nc.scalar.sqrt(sr, onema)
                # Horner: p = (((c7*a + c6)*a + c5)*a + ... + c0)
                poly = pp.tile([128, FF], F32)
                nc.vector.tensor_scalar(
                    poly, av, ACOS_C[7], ACOS_C[6],
                    mybir.AluOpType.mult, mybir.AluOpType.add,
                )
                tmp_m = pp.tile([128, FF], F32)
                for k in range(5, -1, -1):
                    nc.vector.tensor_mul(tmp_m, poly, av)
                    nc.vector.tensor_scalar_add(poly, tmp_m, ACOS_C[k])
                rr = pp.tile([128, FF], F32)
                nc.vector.tensor_mul(rr, sr, poly)
                negm = pp.tile([128, FF], F32)
                nc.vector.tensor_scalar(
                    negm, cc, 0.0, None, mybir.AluOpType.is_lt
                )
                w1 = pp.tile([128, FF], F32)
                nc.vector.tensor_scalar(
                    w1, rr, -2.0, math.pi, mybir.AluOpType.mult, mybir.AluOpType.add
                )
                w2 = pp.tile([128, FF], F32)
                nc.vector.tensor_mul(w2, w1, negm)
                psit = pp.tile([128, FF], F32)
                nc.vector.tensor_add(psit, rr, w2)
                # bounce to DRAM (contiguous: token t = p*FF + f) -> [nrows, TB]
                nc.sync.dma_start(
                    out=rowsd[2].rearrange("(p f) -> p f", p=128), in_=psit
                )

# angle rows stay in DRAM (posp / rowsd[2]); [1, TB] slices are
            # staged into SBUF per token-block inside the phase-B loop.

            # ---- resident rope'd activations ----
            krt = res.tile([128, OC, n], F16)      # K_rot^T
            qrt = res.tile([128, OC, chunk], F16)  # Q_rot^T

            # ================= phase B: projections + rope =================
            with (
                tc.tile_pool(name="xt_p", bufs=2) as xt_p,
                tc.tile_pool(name="cs_p", bufs=1) as cs_p,
                tc.tile_pool(name="rtmp", bufs=2) as rtmp,
                tc.tile_pool(name="vs_p", bufs=2) as vs_p,
                tc.tile_pool(name="fq_ps", bufs=2, space="PSUM") as fq_ps,
                tc.tile_pool(name="kq_ps", bufs=2, space="PSUM") as kq_ps,
                tc.tile_pool(name="v_ps", bufs=1, space="PSUM") as v_ps,
            ):
                for tb in range(n_tb):
                    t0 = tb * TB
                    xt = xt_p.tile([128, OC, TB], F16)
                    for ic in range(OC):
                        nc.sync.dma_start(
                            out=xt[:, ic, :],
                            in_=x16d[t0:t0 + TB, ic * 128:(ic + 1) * 128],
                            transpose=True,
                        )
                    coss, sins = [], []
                    for ch in range(3):
                        arow = cs_p.tile([1, TB], F32, tag=f"arow{ch}")
                        if ch == 2:
                            src = rowsd[2][t0:t0 + TB].unsqueeze(0)
                        else:
                            src = posp[ch, tb].unsqueeze(0)
                        nc.sync.dma_start(out=arow, in_=src)
                        fq = fq_ps.tile([128, TB], F32, tag="fq")
                        nc.tensor.matmul(
                            fq, lhsT=invf_s, rhs=arow, start=True, stop=True,
                        )
                        # range-reduce to [-pi, pi] (freqs are in [0, 2pi]):
                        # fs = fq - 2pi*(fq > pi); cos(fs) = sin(pi/2 - |fs|)
                        msk = rtmp.tile([128, TB], F32, tag="m")
                        nc.vector.tensor_scalar(
                            msk, fq, math.pi, None, mybir.AluOpType.is_gt
                        )
                        fs = cs_p.tile([128, TB], F32, tag="fs", bufs=2)
                        nc.vector.scalar_tensor_tensor(
                            fs, msk, -2.0 * math.pi, fq,
                            mybir.AluOpType.mult, mybir.AluOpType.add,
                        )
                        sinc = cs_p.tile([128, TB], F16, tag=f"sin{ch}")
                        nc.scalar.activation(
                            sinc, fs, mybir.ActivationFunctionType.Sin
                        )
                        afs = rtmp.tile([128, TB], F32, tag="m")
                        nc.scalar.activation(
                            afs, fs, mybir.ActivationFunctionType.Abs
                        )
                        cosc = cs_p.tile([128, TB], F16, tag=f"cos{ch}")
                        nc.scalar.activation(
                            cosc, afs, mybir.ActivationFunctionType.Sin,
                            bias=halfpi[:, 0:1], scale=-1.0,
                        )
                        coss.append(cosc)
                        sins.append(sinc)

                    # K^T (and Q^T when in chunk) in rope pairs (oc, oc+3)
                    qn = min(TB, chunk - t0)  # Q tokens in this block (<=0: none)
                    targets = [(1, bk_s, krt, slice(t0, t0 + TB), TB)]
                    if qn > 0:
                        targets.append((0, bq_s, qrt, slice(t0, t0 + qn), qn))
                    for wsel, bias_s, dst, dslice, wd in targets:
                        wti = wt[wsel]
                        for pr in range(3):
                            o1, o2 = pr, pr + 3
                            p1 = kq_ps.tile([128, TB], F32, tag="p1")
                            p2 = kq_ps.tile([128, TB], F32, tag="p2")
                            for ic in range(OC):
                                nc.tensor.matmul(
                                    p1[:, :wd],
                                    lhsT=wti[:, ic, o1 * 128:(o1 + 1) * 128],
                                    rhs=xt[:, ic, :wd],
                                    start=(ic == 0), stop=(ic == OC - 1),
                                )
                            for ic in range(OC):
                                nc.tensor.matmul(
                                    p2[:, :wd],
                                    lhsT=wti[:, ic, o2 * 128:(o2 + 1) * 128],
                                    rhs=xt[:, ic, :wd],
                                    start=(ic == 0), stop=(ic == OC - 1),
                                )
                            a_t = rtmp.tile([128, TB], F32, tag="a")
                            b_t = rtmp.tile([128, TB], F32, tag="b")
                            nc.vector.scalar_tensor_tensor(
                                a_t[:, :wd], p1[:, :wd], bias_s[:, o1:o1 + 1],
                                coss[pr][:, :wd],
                                mybir.AluOpType.add, mybir.AluOpType.mult,
                            )
                            nc.vector.scalar_tensor_tensor(
                                b_t[:, :wd], p2[:, :wd], bias_s[:, o2:o2 + 1],
                                sins[pr][:, :wd],
                                mybir.AluOpType.add, mybir.AluOpType.mult,
                            )
                            nc.vector.tensor_sub(
                                dst[:, o1, dslice], a_t[:, :wd], b_t[:, :wd]
                            )
                            c_t = rtmp.tile([128, TB], F32, tag="a")
                            d_t = rtmp.tile([128, TB], F32, tag="b")
                            nc.vector.scalar_tensor_tensor(
                                c_t[:, :wd], p1[:, :wd], bias_s[:, o1:o1 + 1],
                                sins[pr][:, :wd],
                                mybir.AluOpType.add, mybir.AluOpType.mult,
                            )
                            nc.vector.scalar_tensor_tensor(
                                d_t[:, :wd], p2[:, :wd], bias_s[:, o2:o2 + 1],
                                coss[pr][:, :wd],
                                mybir.AluOpType.add, mybir.AluOpType.mult,
                            )
                            nc.vector.tensor_add(
                                dst[:, o2, dslice], c_t[:, :wd], d_t[:, :wd]
                            )

                    # V natural [t, d] -> DRAM fp16
                    for t4 in range(TB // 128):
                        vs = vs_p.tile([128, D], F16, tag="vs")
                        for half in range(2):
                            nsl = slice(half * 384, (half + 1) * 384)
                            vp = v_ps.tile([128, 384], F32, tag=f"vp{half}",
                                           name=f"vp{half}")
                            for ic in range(OC):
                                nc.tensor.matmul(
                                    vp,
                                    lhsT=xt[:, ic, t4 * 128:(t4 + 1) * 128],
                                    rhs=wt[2][:, ic, nsl],
                                    start=(ic == 0), stop=(ic == OC - 1),
                                )
                            nc.vector.scalar_tensor_tensor(
                                vs[:, nsl], vp, 0.0, bv_b[:, nsl],
                                mybir.AluOpType.bypass, mybir.AluOpType.add,
                            )
                        nc.sync.dma_start(
                            out=v16d[t0 + t4 * 128:t0 + (t4 + 1) * 128, :], in_=vs
                        )

            # ================= phase C: attention =================
            with (
                tc.tile_pool(name="vt_p", bufs=4) as vt_p,
                tc.tile_pool(name="pt_p", bufs=2) as pt_p,
                tc.tile_pool(name="ep_p", bufs=1) as ep_p,
                tc.tile_pool(name="ob_p", bufs=2) as ob_p,
                tc.tile_pool(name="u_ps", bufs=1, space="PSUM") as u_ps,
                tc.tile_pool(name="l_ps", bufs=1, space="PSUM") as l_ps,
                tc.tile_pool(name="s_ps", bufs=1, space="PSUM") as s_ps,
            ):
                bo_b = ep_p.tile([128, D], F32, tag="bo_b")
                nc.gpsimd.dma_start(out=bo_b, in_=bor[:].broadcast_to([128, D]))
                nlc = qb_size // 128
                for qb in range(n_qb):
                    q0 = qb * qb_size
                    ups = [
                        u_ps.tile([128, qb_size], F32, tag=f"u{oc}",
                                  name=f"u{oc}")
                        for oc in range(OC)
                    ]
                    lp = l_ps.tile([1, qb_size], F32, tag="l")
                    prev = None  # (pt, vt) of previous k-tile
                    for kt in range(n_kt):
                        sp = s_ps.tile([128, qb_size], F32, tag="sx")
                        for ic in range(OC):
                            nc.tensor.matmul(
                                sp, lhsT=krt[:, ic, kt * 128:(kt + 1) * 128],
                                rhs=qrt[:, ic, q0:q0 + qb_size],
                                start=(ic == 0), stop=(ic == OC - 1),
                            )
                        pt = pt_p.tile([128, qb_size], F16, tag="pt")
                        nc.scalar.activation(
                            pt, sp, mybir.ActivationFunctionType.Exp, scale=SCALE
                        )
                        vt = vt_p.tile([128, D], F16, tag="vt")
                        nc.sync.dma_start(
                            out=vt, in_=v16d[kt * 128:(kt + 1) * 128, :]
                        )
                        # U/l matmuls for the *previous* k-tile run while ACT
                        # computes exp(kt) -> no PE bubble with 1 sp buffer.
                        def u_l(pt_, vt_, kti):
                            st = (kti == 0)
                            sp_ = (kti == n_kt - 1)
                            for oc in range(OC):
                                nc.tensor.matmul(
                                    ups[oc],
                                    lhsT=vt_[:, oc * 128:(oc + 1) * 128],
                                    rhs=pt_, start=st, stop=sp_,
                                )
                            nc.tensor.matmul(
                                lp, lhsT=ones16, rhs=pt_, start=st, stop=sp_,
                            )
                        if prev is not None:
                            u_l(*prev)
                        prev = (pt, vt, kt)
                    u_l(*prev)

                    # epilogue: 1/l, U^T -> SBUF fp16, out-projection
                    lrow = ep_p.tile([1, qb_size], F32, tag="lrow")
                    nc.scalar.copy(lrow, lp)
                    ldram = dram.tile([qb_size], F32, tag="ldram", name="ldram")
                    nc.sync.dma_start(out=ldram.unsqueeze(0), in_=lrow)
                    rec_in = ep_p.tile([128, nlc], F32, tag="rec_in")
                    nc.sync.dma_start(
                        out=rec_in,
                        in_=ldram.rearrange("(tj p) -> p tj", p=128),
                    )
                    rec = ep_p.tile([128, nlc], F32, tag="rec")
                    nc.vector.reciprocal(rec, rec_in)
                    us = []
                    for oc in range(OC):
                        ut = ep_p.tile([128, qb_size], F16, tag=f"us{oc}")
                        nc.scalar.copy(ut, ups[oc])
                        us.append(ut)
                    for tj in range(nlc):
                        op_ = s_ps.tile([128, 384], F32, tag="sx")
                        ob = ob_p.tile([128, D], F32, tag="ob")
                        for half in range(2):
                            nsl = slice(half * 384, (half + 1) * 384)
                            for oc in range(OC):
                                nc.tensor.matmul(
                                    op_,
                                    lhsT=us[oc][:, tj * 128:(tj + 1) * 128],
                                    rhs=wt[3][:, oc, nsl],
                                    start=(oc == 0), stop=(oc == OC - 1),
                                )
                            nc.vector.scalar_tensor_tensor(
                                ob[:, nsl], op_, rec[:, tj:tj + 1], bo_b[:, nsl],
                                mybir.AluOpType.mult, mybir.AluOpType.add,
                            )
                            if half == 0:
                                op_ = s_ps.tile([128, 384], F32, tag="sx")
                        nc.sync.dma_start(
                            out=out[q0 + tj * 128:q0 + (tj + 1) * 128, :], in_=ob
                        )

    nc.finalize()
    return nc


def _in_map(x_b, pos_b, vd_b, ref_b, weights, n, roll):
    """Build one core's input map; roll token axis so q-chunk = rows 0:CHUNK."""
    xr = np.roll(x_b, -roll, axis=0)
    posr = np.roll(pos_b, -roll, axis=0)
    vdr = np.roll(vd_b, -roll, axis=0)
    wq, bq, wk, bk, wv, bv, wo, bo = weights
    nrows = n // TB
    inv_freq = (1.0 / (ROPE_BASE ** (np.arange(PER, dtype=np.float64) / PER)))
    return {
        "x": np.ascontiguousarray(xr, dtype=np.float32),
        "vdp": np.ascontiguousarray(
            vdr.reshape(128, (n // 128) * 3, order="C"), dtype=np.float32
        ),
        "posp": np.ascontiguousarray(
            posr.T.reshape(2, nrows, TB), dtype=np.float32
        ),
        "ref": np.ascontiguousarray(ref_b.reshape(1, 3), dtype=np.float32),
        "Wq": np.ascontiguousarray(wq, dtype=np.float32),
        "Wk": np.ascontiguousarray(wk, dtype=np.float32),
        "Wv": np.ascontiguousarray(wv, dtype=np.float32),
        "Wo": np.ascontiguousarray(wo, dtype=np.float32),
        "bqc": np.ascontiguousarray(bq.reshape(OC, 128).T, dtype=np.float32),
        "bkc": np.ascontiguousarray(bk.reshape(OC, 128).T, dtype=np.float32),
        "bvr": np.ascontiguousarray(bv.reshape(1, D), dtype=np.float32),
        "bor": np.ascontiguousarray(bo.reshape(1, D), dtype=np.float32),
        "invf": np.ascontiguousarray(inv_freq.reshape(1, PER), dtype=np.float32),
    }


_NC_CACHE = {}


def _run(inputs, **spmd_kwargs):
    x = np.asarray(inputs["x"]); pos = np.asarray(inputs["pos"])
    vd = np.asarray(inputs["vd"]); ref = np.asarray(inputs["ref"])
    weights = tuple(
        np.asarray(inputs[k])
        for k in ("Wq", "bq", "Wk", "bk", "Wv", "bv", "Wo", "bo")
    )

    if "nc" not in _NC_CACHE:
        _NC_CACHE["nc"] = build_nc(N, CHUNK)
    nc = _NC_CACHE["nc"]

    in_maps = []
    for c in range(NCORES):
        b, r = divmod(c, SPLIT)
        in_maps.append(
            _in_map(x[b], pos[b], vd[b], ref[b], weights, N, r * CHUNK)
        )
    res = run_bass_kernel_spmd(
        nc, in_maps, core_ids=list(range(NCORES)), **spmd_kwargs
    )
    outp = np.empty((B, N, D), dtype=np.float32)
    for c in range(NCORES):
        b, r = divmod(c, SPLIT)
        outp[b, r * CHUNK:(r + 1) * CHUNK] = res.results[c]["out"]
    return outp, res


def kernel(x, pos, vd, ref, Wq, bq, Wk, bk, Wv, bv, Wo, bo):
    outp, _ = _run(dict(x=x, pos=pos, vd=vd, ref=ref, Wq=Wq, bq=bq, Wk=Wk,
                        bk=bk, Wv=Wv, bv=bv, Wo=Wo, bo=bo))
    return outp


# revision 34
# speedup vs baseline: 1.0857x; 1.0857x over previous
"""Trainium2 Bass kernel for nn_MiniAttn (B=2, N=8192, D=768 attention w/ S2 rope).

Sharding: 8 cores = 2 batches x 4 q-chunks of 2048 rows. Each core gets its
batch's tokens *rolled* so its q-chunk is always rows 0:2048 (softmax over
keys is permutation invariant), letting one SPMD program serve all cores.

Device algorithm (per core):
  phase A: cast x/W to fp16 in DRAM, DMA-transpose weights to SBUF,
           compute psi=arccos(vdn.refn) on-chip, build theta/phi/psi rows.
  phase B: per 512-token block: DMA-transpose x^T slice, project K^T/Q^T/V
           (fp16 matmuls), apply rope via sin/cos built from PE outer
           products + ACT Sin; K_rot^T and Q_rot^T stay resident in SBUF,
           V streams to DRAM.
  phase C: flash-attention without max-subtraction (scores ~N(0,1)):
           S^T tile = K_rot^T.T @ Q_rot^T, P=exp(S/sqrt(D)) (ACT, fp16),
           U^T += V.T @ P per k-tile (PSUM accum), l += P.T @ ones,
           then out = (U^T.T @ Wo^T) * (1/l) + bo.
"""

import math
import sys

import numpy as np

sys.path.insert(0, "/opt/trn_rl_repo")

import concourse.bass as bass  # noqa: E402
import concourse.mybir as mybir  # noqa: E402
import concourse.tile as tile  # noqa: E402
from concourse import bacc  # noqa: E402
from concourse.bass_utils import run_bass_kernel_spmd  # noqa: E402

F32 = mybir.dt.float32
F16 = mybir.dt.float16
F32R = mybir.dt.float32r

B, N, D = 2, 8192, 768
NCORES = 8
SPLIT = NCORES // B          # q-chunks per batch
CHUNK = N // SPLIT           # q rows per core
ROPE_BASE = 10000.0
NPAIR = D // 2               # 384 rotary pairs
PER = NPAIR // 3             # 128 pairs per angle channel
OC = D // 128                # 6 output 128-chunks
TB = 512                     # token block in phase B
SCALE = 1.0 / math.sqrt(D)

# arccos(x) ~= sqrt(1-x) * poly(x) on [0,1]  (Abramowitz & Stegun 4.4.46,
# |err| <= 2e-8); arccos(-x) = pi - arccos(x)
ACOS_C = [1.5707963050, -0.2145988016, 0.0889789874, -0.0501743046,
          0.0308918810, -0.0170881256, 0.0066700901, -0.0012624911]


def build_nc(n=N, chunk=CHUNK):
    assert n % TB == 0 and chunk % 128 == 0
    qb_size = min(512, chunk)
    n_qb = chunk // qb_size
    n_tb = n // TB
    n_kt = n // 128
    nrows = n // TB          # partitions in the [nrows, TB] angle-row tiles

    nc = bacc.Bacc(None)

    x = nc.declare_dram_parameter("x", [n, D], F32, isOutput=False)
    vdp = nc.declare_dram_parameter("vdp", [128, (n // 128) * 3], F32, isOutput=False)
    posp = nc.declare_dram_parameter("posp", [2, nrows, TB], F32, isOutput=False)
    ref = nc.declare_dram_parameter("ref", [1, 3], F32, isOutput=False)
    wq = nc.declare_dram_parameter("Wq", [D, D], F32, isOutput=False)
    wk = nc.declare_dram_parameter("Wk", [D, D], F32, isOutput=False)
    wv = nc.declare_dram_parameter("Wv", [D, D], F32, isOutput=False)
    wo = nc.declare_dram_parameter("Wo", [D, D], F32, isOutput=False)
    bqc = nc.declare_dram_parameter("bqc", [128, OC], F32, isOutput=False)
    bkc = nc.declare_dram_parameter("bkc", [128, OC], F32, isOutput=False)
    bvr = nc.declare_dram_parameter("bvr", [1, D], F32, isOutput=False)
    bor = nc.declare_dram_parameter("bor", [1, D], F32, isOutput=False)
    invf = nc.declare_dram_parameter("invf", [1, PER], F32, isOutput=False)
    out = nc.declare_dram_parameter("out", [chunk, D], F32, isOutput=True)

    ws = [wq, wk, wv, wo]

    with tile.TileContext(nc) as tc:
        with (
            tc.tile_pool(name="dram", bufs=1, space="DRAM") as dram,
            tc.tile_pool(name="consts", bufs=1) as consts,
            tc.tile_pool(name="resident", bufs=1) as res,
        ):
            # ---- DRAM scratch ----
            x16d = dram.tile([n, D], F16)
            w16d = dram.tile([4, D, D], F16)
            v16d = dram.tile([n, D], F16)
            rowsd = dram.tile([3, n], F32)

            # fp32 -> fp16 casts (SWDGE DMA casts in flight); x cast is
            # chunked per token-block so phase B can start on early blocks.
            for i in range(4):
                nc.gpsimd.dma_start(out=w16d[i], in_=ws[i][:])
            for tbc in range(n_tb):
                tc0 = tbc * TB
                nc.gpsimd.dma_start(
                    out=x16d[tc0:tc0 + TB, :], in_=x[tc0:tc0 + TB, :]
                )

            # ---- weights transposed into SBUF: wt[w][:, ic, :] = W^T[in-chunk ic]
            wt = []
            for wi in range(4):
                wtile = res.tile([128, OC, D], F16, name=f"wt{wi}")
                for ic in range(OC):
                    nc.sync.dma_start(
                        out=wtile[:, ic, :],
                        in_=w16d[wi, :, ic * 128:(ic + 1) * 128],
                        transpose=True,
                    )
                wt.append(wtile)

            # ---- small consts ----
            bq_s = consts.tile([128, OC], F32)
            nc.sync.dma_start(out=bq_s, in_=bqc[:])
            bk_s = consts.tile([128, OC], F32)
            nc.sync.dma_start(out=bk_s, in_=bkc[:])
            bv_b = consts.tile([128, D], F32)
            nc.gpsimd.dma_start(out=bv_b, in_=bvr[:].broadcast_to([128, D]))
            invf_s = consts.tile([1, PER], F32)
            nc.sync.dma_start(out=invf_s, in_=invf[:])
            ones16 = consts.tile([128, 1], F16)
            nc.vector.memset(ones16, 1.0)
            halfpi = consts.tile([128, 1], F32)
            nc.vector.memset(halfpi, math.pi / 2)

            # ---- psi = arccos(clip(vdn . refn)) in [128, n/128] p-major layout
            FF = n // 128
            with tc.tile_pool(name="psip", bufs=1) as pp:
                r13 = pp.tile([1, 3], F32)
                nc.sync.dma_start(out=r13, in_=ref[:])
                rsq = pp.tile([1, 3], F32)
                nc.vector.tensor_mul(rsq, r13, r13)
                rs1 = pp.tile([1, 1], F32)
                nc.vector.reduce_sum(rs1, rsq, axis=mybir.AxisListType.X)
                rsr = pp.tile([1, 1], F32)
                nc.scalar.sqrt(rsr, rs1)
                rrec = pp.tile([1, 1], F32)
                nc.vector.reciprocal(rrec, rsr)
                r13n = pp.tile([1, 3], F32)
                nc.vector.tensor_scalar(r13n, r13, rrec, None, mybir.AluOpType.mult)
                refd = dram.tile([1, 3], F32)
                nc.sync.dma_start(out=refd, in_=r13n)
                refb = pp.tile([128, 3], F32)
                nc.gpsimd.dma_start(out=refb, in_=refd[:].broadcast_to([128, 3]))

                vdt = pp.tile([128, FF, 3], F32)
                nc.sync.dma_start(out=vdt, in_=vdp[:].rearrange("p (f c) -> p f c", c=3))
                sq3 = pp.tile([128, FF, 3], F32)
                nc.vector.tensor_mul(sq3, vdt, vdt)
                nrm = pp.tile([128, FF], F32)
                nc.vector.reduce_sum(nrm, sq3, axis=mybir.AxisListType.X)
                srt = pp.tile([128, FF], F32)
                nc.scalar.sqrt(srt, nrm)
                rsn = pp.tile([128, FF], F32)
                nc.vector.reciprocal(rsn, srt)
                prod = pp.tile([128, FF, 3], F32)
                nc.vector.tensor_mul(
                    prod, vdt, refb[:, None, :].broadcast_to([128, FF, 3])
                )
                dotv = pp.tile([128, FF], F32)
                nc.vector.reduce_sum(dotv, prod, axis=mybir.AxisListType.X)
                cc = pp.tile([128, FF], F32)
                nc.vector.tensor_mul(cc, dotv, rsn)
                nc.vector.tensor_scalar_min(cc, cc, 1.0)
                nc.vector.tensor_scalar_max(cc, cc, -1.0)
                av = pp.tile([128, FF], F32)
                nc.scalar.activation(av, cc, mybir.ActivationFunctionType.Abs)
                onema = pp.tile([128, FF], F32)
                nc.vector.tensor_scalar(
                    onema, av, -1.0, 1.0, mybir.AluOpType.mult, mybir.AluOpType.add
                )
                sr = pp.tile([128, FF], F32)
                nc.scalar.sqrt(sr, onema)
                # Horner: p = (((c7*a + c6)*a + c5)*a + ... + c0)
                poly = pp.tile([128, FF], F32)
                nc.vector.tensor_scalar(
                    poly, av, ACOS_C[7], ACOS_C[6],
                    mybir.AluOpType.mult, mybir.AluOpType.add,
                )
                tmp_m = pp.tile([128, FF], F32)
                for k in range(5, -1, -1):
                    nc.vector.tensor_mul(tmp_m, poly, av)
                    nc.vector.tensor_scalar_add(poly, tmp_m, ACOS_C[k])
                rr = pp.tile([128, FF], F32)
                nc.vector.tensor_mul(rr, sr, poly)
                negm = pp.tile([128, FF], F32)
                nc.vector.tensor_scalar(
                    negm, cc, 0.0, None, mybir.AluOpType.is_lt
                )
                w1 = pp.tile([128, FF], F32)
                nc.vector.tensor_scalar(
                    w1, rr, -2.0, math.pi, mybir.AluOpType.mult, mybir.AluOpType.add
                )
                w2 = pp.tile([128, FF], F32)
                nc.vector.tensor_mul(w2, w1, negm)
                psit = pp.tile([128, FF], F32)
                nc.vector.tensor_add(psit, rr, w2)
                # bounce to DRAM (contiguous: token t = p*FF + f) -> [nrows, TB]
                nc.sync.dma_start(
                    out=rowsd[2].rearrange("(p f) -> p f", p=128), in_=psit
                )

# angle rows stay in DRAM (posp / rowsd[2]); [1, TB] slices are
            # staged into SBUF per token-block inside the phase-B loop.

            # ---- resident rope'd activations ----
            krt = res.tile([128, OC, n], F16)      # K_rot^T
            qrt = res.tile([128, OC, chunk], F16)  # Q_rot^T

            # ================= phase B: projections + rope =================
            with (
                tc.tile_pool(name="xt_p", bufs=2) as xt_p,
                tc.tile_pool(name="cs_p", bufs=1) as cs_p,
                tc.tile_pool(name="rtmp", bufs=2) as rtmp,
                tc.tile_pool(name="vs_p", bufs=2) as vs_p,
                tc.tile_pool(name="fq_ps", bufs=2, space="PSUM") as fq_ps,
                tc.tile_pool(name="kq_ps", bufs=2, space="PSUM") as kq_ps,
                tc.tile_pool(name="v_ps", bufs=1, space="PSUM") as v_ps,
            ):
                for tb in range(n_tb):
                    t0 = tb * TB
                    xt = xt_p.tile([128, OC, TB], F16)
                    for ic in range(OC):
                        nc.sync.dma_start(
                            out=xt[:, ic, :],
                            in_=x16d[t0:t0 + TB, ic * 128:(ic + 1) * 128],
                            transpose=True,
                        )
                    coss, sins = [], []
                    for ch in range(3):
                        arow = cs_p.tile([1, TB], F32, tag=f"arow{ch}")
                        if ch == 2:
                            src = rowsd[2][t0:t0 + TB].unsqueeze(0)
                        else:
                            src = posp[ch, tb].unsqueeze(0)
                        nc.sync.dma_start(out=arow, in_=src)
                        fq = fq_ps.tile([128, TB], F32, tag="fq")
                        nc.tensor.matmul(
                            fq, lhsT=invf_s, rhs=arow, start=True, stop=True,
                        )
                        # range-reduce to [-pi, pi] (freqs are in [0, 2pi]):
                        # fs = fq - 2pi*(fq > pi); cos(fs) = sin(pi/2 - |fs|)
                        msk = rtmp.tile([128, TB], F32, tag="m")
                        nc.vector.tensor_scalar(
                            msk, fq, math.pi, None, mybir.AluOpType.is_gt
                        )
                        fs = cs_p.tile([128, TB], F32, tag="fs", bufs=2)
                        nc.vector.scalar_tensor_tensor(
                            fs, msk, -2.0 * math.pi, fq,
                            mybir.AluOpType.mult, mybir.AluOpType.add,
                        )
                        sinc = cs_p.tile([128, TB], F16, tag=f"sin{ch}")
                        nc.scalar.activation(
                            sinc, fs, mybir.ActivationFunctionType.Sin
                        )
                        afs = rtmp.tile([128, TB], F32, tag="m")
                        nc.scalar.activation(
                            afs, fs, mybir.ActivationFunctionType.Abs
                        )
                        cosc = cs_p.tile([128, TB], F16, tag=f"cos{ch}")
                        nc.scalar.activation(
                            cosc, afs, mybir.ActivationFunctionType.Sin,
                            bias=halfpi[:, 0:1], scale=-1.0,
                        )
                        coss.append(cosc)
                        sins.append(sinc)

                    # K^T (and Q^T when in chunk) in rope pairs (oc, oc+3)
                    qn = min(TB, chunk - t0)  # Q tokens in this block (<=0: none)
                    targets = [(1, bk_s, krt, slice(t0, t0 + TB), TB)]
                    if qn > 0:
                        targets.append((0, bq_s, qrt, slice(t0, t0 + qn), qn))
                    for wsel, bias_s, dst, dslice, wd in targets:
                        wti = wt[wsel]
                        for pr in range(3):
                            o1, o2 = pr, pr + 3
                            p1 = kq_ps.tile([128, TB], F32, tag="p1")
                            p2 = kq_ps.tile([128, TB], F32, tag="p2")
                            for ic in range(OC):
                                nc.tensor.matmul(
                                    p1[:, :wd],
                                    lhsT=wti[:, ic, o1 * 128:(o1 + 1) * 128],
                                    rhs=xt[:, ic, :wd],
                                    start=(ic == 0), stop=(ic == OC - 1),
                                )
                            for ic in range(OC):
                                nc.tensor.matmul(
                                    p2[:, :wd],
                                    lhsT=wti[:, ic, o2 * 128:(o2 + 1) * 128],
                                    rhs=xt[:, ic, :wd],
                                    start=(ic == 0), stop=(ic == OC - 1),
                                )
# PSUM -> SBUF fp16 with bias on ACT (per-partition bias),
                            # then all-fp16 rope arithmetic on DVE (2x mode)
                            k1s = rtmp.tile([128, TB], F16, tag="k1")
                            nc.scalar.activation(
                                k1s[:, :wd], p1[:, :wd],
                                mybir.ActivationFunctionType.Identity,
                                bias=bias_s[:, o1:o1 + 1],
                            )
                            k2s = rtmp.tile([128, TB], F16, tag="k2")
                            nc.scalar.activation(
                                k2s[:, :wd], p2[:, :wd],
                                mybir.ActivationFunctionType.Identity,
                                bias=bias_s[:, o2:o2 + 1],
                            )
                            a_t = rtmp.tile([128, TB], F16, tag="a")
                            b_t = rtmp.tile([128, TB], F16, tag="b")
                            nc.vector.tensor_mul(
                                a_t[:, :wd], k1s[:, :wd], coss[pr][:, :wd]
                            )
                            nc.vector.tensor_mul(
                                b_t[:, :wd], k2s[:, :wd], sins[pr][:, :wd]
                            )
                            nc.vector.tensor_sub(
                                dst[:, o1, dslice], a_t[:, :wd], b_t[:, :wd]
                            )
                            c_t = rtmp.tile([128, TB], F16, tag="a")
                            d_t = rtmp.tile([128, TB], F16, tag="b")
                            nc.vector.tensor_mul(
                                c_t[:, :wd], k1s[:, :wd], sins[pr][:, :wd]
                            )
                            nc.vector.tensor_mul(
                                d_t[:, :wd], k2s[:, :wd], coss[pr][:, :wd]
                            )
                            nc.vector.tensor_add(
                                dst[:, o2, dslice], c_t[:, :wd], d_t[:, :wd]
                            )

                    # V natural [t, d] -> DRAM fp16
                    for t4 in range(TB // 128):
                        vs = vs_p.tile([128, D], F16, tag="vs")
                        for half in range(2):
                            nsl = slice(half * 384, (half + 1) * 384)
                            vp = v_ps.tile([128, 384], F32, tag=f"vp{half}",
                                           name=f"vp{half}")
                            for ic in range(OC):
                                nc.tensor.matmul(
                                    vp,
                                    lhsT=xt[:, ic, t4 * 128:(t4 + 1) * 128],
                                    rhs=wt[2][:, ic, nsl],
                                    start=(ic == 0), stop=(ic == OC - 1),
                                )
                            nc.vector.scalar_tensor_tensor(
                                vs[:, nsl], vp, 0.0, bv_b[:, nsl],
                                mybir.AluOpType.bypass, mybir.AluOpType.add,
                            )
                        nc.sync.dma_start(
                            out=v16d[t0 + t4 * 128:t0 + (t4 + 1) * 128, :], in_=vs
                        )

            # ================= phase C: attention =================
            with (
                tc.tile_pool(name="vt_p", bufs=4) as vt_p,
                tc.tile_pool(name="pt_p", bufs=2) as pt_p,
                tc.tile_pool(name="ep_p", bufs=1) as ep_p,
                tc.tile_pool(name="ob_p", bufs=2) as ob_p,
                tc.tile_pool(name="u_ps", bufs=1, space="PSUM") as u_ps,
                tc.tile_pool(name="l_ps", bufs=1, space="PSUM") as l_ps,
                tc.tile_pool(name="s_ps", bufs=1, space="PSUM") as s_ps,
            ):
                bo_b = ep_p.tile([128, D], F32, tag="bo_b")
                nc.gpsimd.dma_start(out=bo_b, in_=bor[:].broadcast_to([128, D]))
                nlc = qb_size // 128
                for qb in range(n_qb):
                    q0 = qb * qb_size
                    ups = [
                        u_ps.tile([128, qb_size], F32, tag=f"u{oc}",
                                  name=f"u{oc}")
                        for oc in range(OC)
                    ]
                    lp = l_ps.tile([1, qb_size], F32, tag="l")
                    G = 4 if n_kt % 4 == 0 else 1  # l-accumulation batch
                    ptacc = None
                    prev = None  # (pt, vt) of previous k-tile
                    for kt in range(n_kt):
                        sp = s_ps.tile([128, qb_size], F32, tag="sx")
                        for ic in range(OC):
                            nc.tensor.matmul(
                                sp, lhsT=krt[:, ic, kt * 128:(kt + 1) * 128],
                                rhs=qrt[:, ic, q0:q0 + qb_size],
                                start=(ic == 0), stop=(ic == OC - 1),
                            )
                        pt = pt_p.tile([128, qb_size], F16, tag="pt")
                        nc.scalar.activation(
                            pt, sp, mybir.ActivationFunctionType.Exp, scale=SCALE
                        )
                        vt = vt_p.tile([128, D], F16, tag="vt")
                        nc.sync.dma_start(
                            out=vt, in_=v16d[kt * 128:(kt + 1) * 128, :]
                        )
                        # batch l: sum G exp-tiles on DVE, one PE matmul per G
                        if kt % G == 0:
                            ptacc = pt_p.tile([128, qb_size], F16,
                                              tag="ptacc", name="ptacc")
                            nc.vector.tensor_copy(ptacc, pt)
                        else:
                            nc.vector.tensor_add(ptacc, ptacc, pt)
                        if kt % G == G - 1 or kt == n_kt - 1:
                            nc.tensor.matmul(
                                lp, lhsT=ones16, rhs=ptacc,
                                start=(kt < G), stop=(kt == n_kt - 1),
                            )
                        # U/l matmuls for the *previous* k-tile run while ACT
                        # computes exp(kt) -> no PE bubble with 1 sp buffer.
                        def u_l(pt_, vt_, kti):
                            st = (kti == 0)
                            sp_ = (kti == n_kt - 1)
                            for oc in range(OC):
                                nc.tensor.matmul(
                                    ups[oc],
                                    lhsT=vt_[:, oc * 128:(oc + 1) * 128],
                                    rhs=pt_, start=st, stop=sp_,
                                )
                        if prev is not None:
                            u_l(*prev)
                        prev = (pt, vt, kt)
                    u_l(*prev)

                    # epilogue: 1/l, U^T -> SBUF fp16, out-projection
                    lrow = ep_p.tile([1, qb_size], F32, tag="lrow")
                    nc.scalar.copy(lrow, lp)
                    ldram = dram.tile([qb_size], F32, tag="ldram", name="ldram")
                    nc.sync.dma_start(out=ldram.unsqueeze(0), in_=lrow)
                    rec_in = ep_p.tile([128, nlc], F32, tag="rec_in")
                    nc.sync.dma_start(
                        out=rec_in,
                        in_=ldram.rearrange("(tj p) -> p tj", p=128),
                    )
                    rec = ep_p.tile([128, nlc], F32, tag="rec")
                    nc.vector.reciprocal(rec, rec_in)
                    us = []
                    for oc in range(OC):
                        ut = ep_p.tile([128, qb_size], F16, tag=f"us{oc}")
                        nc.scalar.copy(ut, ups[oc])
                        us.append(ut)
                    for tj in range(nlc):
                        op_ = u_ps.tile([128, 384], F32, tag="u0", name="op_")
                        ob = ob_p.tile([128, D], F32, tag="ob")
                        for half in range(2):
                            nsl = slice(half * 384, (half + 1) * 384)
                            for oc in range(OC):
                                nc.tensor.matmul(
                                    op_,
                                    lhsT=us[oc][:, tj * 128:(tj + 1) * 128],
                                    rhs=wt[3][:, oc, nsl],
                                    start=(oc == 0), stop=(oc == OC - 1),
                                )
                            nc.vector.scalar_tensor_tensor(
                                ob[:, nsl], op_, rec[:, tj:tj + 1], bo_b[:, nsl],
                                mybir.AluOpType.mult, mybir.AluOpType.add,
                            )
                            if half == 0:
                                op_ = u_ps.tile([128, 384], F32, tag="u0",
                                                name="op_")
                        nc.sync.dma_start(
                            out=out[q0 + tj * 128:q0 + (tj + 1) * 128, :], in_=ob
                        )

    nc.finalize()
    return nc


def _in_map(x_b, pos_b, vd_b, ref_b, weights, n, roll):
    """Build one core's input map; roll token axis so q-chunk = rows 0:CHUNK."""
    xr = np.roll(x_b, -roll, axis=0)
    posr = np.roll(pos_b, -roll, axis=0)
    vdr = np.roll(vd_b, -roll, axis=0)
    wq, bq, wk, bk, wv, bv, wo, bo = weights
    nrows = n // TB
    inv_freq = (1.0 / (ROPE_BASE ** (np.arange(PER, dtype=np.float64) / PER)))
    return {
        "x": np.ascontiguousarray(xr, dtype=np.float32),
        "vdp": np.ascontiguousarray(
            vdr.reshape(128, (n // 128) * 3, order="C"), dtype=np.float32
        ),
        "posp": np.ascontiguousarray(
            posr.T.reshape(2, nrows, TB), dtype=np.float32
        ),
        "ref": np.ascontiguousarray(ref_b.reshape(1, 3), dtype=np.float32),
        "Wq": np.ascontiguousarray(wq, dtype=np.float32),
        "Wk": np.ascontiguousarray(wk, dtype=np.float32),
        "Wv": np.ascontiguousarray(wv, dtype=np.float32),
        "Wo": np.ascontiguousarray(wo, dtype=np.float32),
        "bqc": np.ascontiguousarray(bq.reshape(OC, 128).T, dtype=np.float32),
        "bkc": np.ascontiguousarray(bk.reshape(OC, 128).T, dtype=np.float32),
        "bvr": np.ascontiguousarray(bv.reshape(1, D), dtype=np.float32),
        "bor": np.ascontiguousarray(bo.reshape(1, D), dtype=np.float32),
        "invf": np.ascontiguousarray(inv_freq.reshape(1, PER), dtype=np.float32),
    }


_NC_CACHE = {}


def _run(inputs, **spmd_kwargs):
    x = np.asarray(inputs["x"]); pos = np.asarray(inputs["pos"])
    vd = np.asarray(inputs["vd"]); ref = np.asarray(inputs["ref"])
    weights = tuple(
        np.asarray(inputs[k])
        for k in ("Wq", "bq", "Wk", "bk", "Wv", "bv", "Wo", "bo")
    )

    if "nc" not in _NC_CACHE:
        _NC_CACHE["nc"] = build_nc(N, CHUNK)
    nc = _NC_CACHE["nc"]

    in_maps = []
    for c in range(NCORES):
        b, r = divmod(c, SPLIT)
        in_maps.append(
            _in_map(x[b], pos[b], vd[b], ref[b], weights, N, r * CHUNK)
        )
    res = run_bass_kernel_spmd(
        nc, in_maps, core_ids=list(range(NCORES)), **spmd_kwargs
    )
    outp = np.empty((B, N, D), dtype=np.float32)
    for c in range(NCORES):
        b, r = divmod(c, SPLIT)
        outp[b, r * CHUNK:(r + 1) * CHUNK] = res.results[c]["out"]
    return outp, res


def kernel(x, pos, vd, ref, Wq, bq, Wk, bk, Wv, bv, Wo, bo):
    outp, _ = _run(dict(x=x, pos=pos, vd=vd, ref=ref, Wq=Wq, bq=bq, Wk=Wk,
                        bk=bk, Wv=Wv, bv=bv, Wo=Wo, bo=bo))
    return outp
